# revision 1
# baseline (speedup 1.0000x reference)
"""GNN message-passing layer on 8 Trainium2 NeuronCores.

Strategy (edge-type sharding, one edge type per core):
  core e: proj_e = node_states @ W[e].T + b[e]            (PE matmul, [N, D])
          gathered = proj_e[src[e]]                        (dma_gather, SWDGE)
          partial_e[tgt[e]] += gathered                    (dma_scatter_add)
  host:   messages = sum_e partial_e ; divide by bincount(tgt).

dma_gather / dma_scatter_add take int16 indices, so nodes are split into 4
buckets of 25000; edges are grouped host-side by (src_bucket, tgt_bucket)
into 16 groups and padded to a static chunk schedule (identical across all
cores: SPMD single program). Padding edges gather row 0 of the bucket and
scatter into per-bucket junk rows (88 spare rows after each target bucket).
"""

import numpy as np

import concourse.bacc as bacc
import concourse.bass as bass
import concourse.mybir as mybir
import concourse.tile as tile
from concourse.bass_utils import run_bass_kernel_spmd
from concourse.masks import make_identity

N = 100000   # nodes
D = 128      # hidden
E = 8        # edge types == cores
M = 200000   # edges per type

NB = 4         # node buckets (int16 index windows)
BS = 25000     # bucket size
JUNK = 88      # junk rows appended to each target bucket (absorb padding)
TBS = BS + JUNK
CHUNK = 1024   # max edges per gather/scatter call (HW SWDGE limit)
SUPER = 1024   # nodes per phase-A supertile
E_GROUPS = 16  # (src_bucket, tgt_bucket) groups

F32 = mybir.dt.float32
I16 = mybir.dt.int16

# test-harness knobs (harness calls kernel() with defaults)
TRACE = False
LAST = None


def build_schedule(edge_lists):
    """Group each core's edges by (src_bucket, tgt_bucket); within each group,
    deal a target's edges across different chunks (occurrence rank) so every
    dma_scatter_add call has unique target indices -- the HW DMA engines race
    on read-modify-write of duplicate rows within one call. In-degree > NMAIN
    within a group spills to small per-level overflow chunks.

    Returns (chunks, tot, gsrc_w, gtgt_w); chunks = (sb, tb, size, col_off).
    """
    assert edge_lists.shape == (E, M, 2)
    src = np.asarray(edge_lists[:, :, 0], dtype=np.int64)
    tgt = np.asarray(edge_lists[:, :, 1], dtype=np.int64)
    sb = src // BS
    tb = tgt // BS
    gid = sb * NB + tb                          # [E, M] group id 0..15
    NMAIN = max(4, -(-M // (E_GROUPS * (CHUNK - 160))))  # initial guess

    # occurrence rank of each edge within (core, group, target)
    occ = np.empty((E, M), dtype=np.int64)
    for e in range(E):
        key = gid[e] * (N + 1) + tgt[e]
        order = np.argsort(key, kind="stable")
        sk = key[order]
        run_start = np.empty(M, dtype=bool)
        run_start[0] = True
        run_start[1:] = sk[1:] != sk[:-1]
        starts = np.flatnonzero(run_start)
        run_id = np.cumsum(run_start) - 1
        occ_sorted = np.arange(M) - starts[run_id]
        occ[e, order] = occ_sorted

    while True:
        is_main = occ < NMAIN
        mcount_try = np.zeros((E, NB * NB, NMAIN), dtype=np.int64)
        mch = (occ + tgt) % NMAIN
        for e in range(E):
            np.add.at(mcount_try[e], (gid[e][is_main[e]], mch[e][is_main[e]]), 1)
        if (-(-mcount_try.max(axis=0) // 128) * 128).max() <= CHUNK:
            break
        NMAIN += 1
    mchunk = (occ + tgt) % NMAIN                # main chunk within group
    olevel = occ - NMAIN                        # overflow level (>=0 where not main)
    n_ovf = int(olevel.max()) + 1 if (~is_main).any() else 0

    # per (core, group, main-chunk) counts -> harmonized caps
    mcount = np.zeros((E, NB * NB, NMAIN), dtype=np.int64)
    ocount = np.zeros((E, NB * NB, max(n_ovf, 1)), dtype=np.int64)
    for e in range(E):
        np.add.at(mcount[e], (gid[e][is_main[e]], mchunk[e][is_main[e]]), 1)
        if n_ovf:
            sel = ~is_main[e]
            np.add.at(ocount[e], (gid[e][sel], olevel[e][sel]), 1)
    mcap = -(-mcount.max(axis=0) // 128) * 128            # [G, NMAIN]
    ocap = -(-ocount.max(axis=0) // 128) * 128            # [G, n_ovf]

    # chunk list: interleave tb so consecutive scatters hit different windows
    chunks = []
    col = 0
    rounds = [("m", r) for r in range(NMAIN)] + [("o", r) for r in range(n_ovf)]
    chunk_off = {}                              # (g, kind, r) -> (off, size)
    for kind, r in rounds:
        for t in range(NB):
            for s_ in range(NB):
                g = s_ * NB + t
                size = int(mcap[g, r] if kind == "m" else ocap[g, r])
                if size == 0:
                    continue
                chunks.append((s_, t, size, col))
                chunk_off[(g, kind, r)] = (col, size)
                col += size
    tot = col
    assert tot % 128 == 0

    gsrc = np.zeros((E, tot), dtype=np.int16)
    gtgt = np.zeros((E, tot), dtype=np.int16)
    pad_t = (BS + (np.arange(tot) % JUNK)).astype(np.int16)
    gtgt[:] = pad_t[None, :]

    for e in range(E):
        for g in range(NB * NB):
            for kind, r in rounds:
                if (g, kind, r) not in chunk_off:
                    continue
                off, size = chunk_off[(g, kind, r)]
                if kind == "m":
                    sel = (gid[e] == g) & is_main[e] & (mchunk[e] == r)
                else:
                    sel = (gid[e] == g) & ~is_main[e] & (olevel[e] == r)
                n = int(sel.sum())
                assert n <= size
                gsrc[e, off:off + n] = (src[e, sel] % BS).astype(np.int16)
                gtgt[e, off:off + n] = (tgt[e, sel] % BS).astype(np.int16)

    # wrap [tot] -> [16, tot//16] (element i at (i % 16, i // 16)), then
    # replicate 8x across partition stripes (one copy per GPSIMD core)
    gsrc_w = np.tile(gsrc.reshape(E, -1, 16).transpose(0, 2, 1), (1, 8, 1))
    gtgt_w = np.tile(gtgt.reshape(E, -1, 16).transpose(0, 2, 1), (1, 8, 1))
    return chunks, tot, np.ascontiguousarray(gsrc_w), np.ascontiguousarray(gtgt_w)


def build_bass(chunks, tot):
    nc = bacc.Bacc("TRN2", target_bir_lowering=False)

    x_d = nc.dram_tensor("x", [N, D], F32, kind="ExternalInput")
    wt_d = nc.dram_tensor("wt", [D, D], F32, kind="ExternalInput")     # W_e^T
    bb_d = nc.dram_tensor("bb", [D, D], F32, kind="ExternalInput")     # b_e bcast
    gs_d = nc.dram_tensor("gsrc", [128, tot // 16], I16, kind="ExternalInput")
    gt_d = nc.dram_tensor("gtgt", [128, tot // 16], I16, kind="ExternalInput")
    proj_d = nc.dram_tensor("proj", [N, D], F32)                       # internal
    msg_d = nc.dram_tensor("msg", [NB * TBS, D], F32, kind="ExternalOutput")

    with tile.TileContext(nc) as tc:
        with (
            tc.tile_pool(name="const", bufs=1) as constp,
            tc.tile_pool(name="xin", bufs=3) as xp,
            tc.tile_pool(name="xt", bufs=4) as xtp,
            tc.tile_pool(name="pout", bufs=3) as op,
            tc.tile_pool(name="gat", bufs=4) as gp,
            tc.tile_pool(name="idx", bufs=4) as ip,
            tc.tile_pool(name="ps1", bufs=4, space="PSUM") as ps1,
            tc.tile_pool(name="ps2", bufs=4, space="PSUM") as ps2,
        ):
            ident = constp.tile([128, 128], F32)
            make_identity(nc, ident[:])
            wt_s = constp.tile([D, D], F32)
            nc.sync.dma_start(wt_s[:], wt_d[:])
            bb_s = constp.tile([D, D], F32)
            nc.sync.dma_start(bb_s[:], bb_d[:])

            # ---- Phase A: proj = x @ W^T + b, 1024-node supertiles ----
            for n0 in range(0, N, SUPER):
                ns = min(SUPER, N - n0)
                nblk = -(-ns // 128)
                full = ns // 128
                rem = ns % 128
                xb = xp.tile([128, nblk, D], F32, tag="xin")
                # x[n0:n0+ns] viewed as [128, nblk, D] (node = n0 + c*128 + p)
                if full:
                    nc.sync.dma_start(
                        xb[:, :full, :],
                        x_d[n0:n0 + full * 128, :].rearrange(
                            "(c p) d -> p c d", p=128
                        ),
                    )
                if rem:
                    # ragged tail rows loaded separately into last block
                    nc.sync.dma_start(
                        xb[:rem, full, :],
                        x_d[n0 + full * 128:n0 + ns, :],
                    )
                ob = op.tile([128, nblk, D], F32, tag="pout")
                for c in range(nblk):
                    bp = 128 if (c + 1) * 128 <= ns else ns - c * 128
                    p1 = ps1.tile([128, 128], F32, tag="ps1")
                    nc.tensor.transpose(p1[:, :bp], xb[:bp, c, :], ident[:bp, :bp])
                    xt = xtp.tile([128, 128], F32, tag="xt")
                    nc.vector.tensor_copy(xt[:, :bp], p1[:, :bp])
                    p2 = ps2.tile([128, D], F32, tag="ps2")
                    nc.tensor.matmul(p2[:bp, :], xt[:, :bp], wt_s[:])
                    nc.vector.tensor_add(ob[:bp, c, :], p2[:bp, :], bb_s[:bp, :])
                if full:
                    nc.sync.dma_start(
                        proj_d[n0:n0 + full * 128, :].rearrange(
                            "(c p) d -> p c d", p=128
                        ),
                        ob[:, :full, :],
                    )
                if rem:
                    nc.sync.dma_start(
                        proj_d[n0 + full * 128:n0 + ns, :], ob[:rem, full, :]
                    )

            # ---- Phase B/C: gather from proj by src, scatter-add to msg ----
            for (sbk, tbk, size, off) in chunks:
                si = ip.tile([128, size // 16], I16, tag="sidx")
                nc.sync.dma_start(si[:], gs_d[:, off // 16:(off + size) // 16])
                ti = ip.tile([128, size // 16], I16, tag="tidx")
                nc.sync.dma_start(ti[:], gt_d[:, off // 16:(off + size) // 16])
                g = gp.tile([128, size // 128, D], F32, tag="gat")
                nc.gpsimd.dma_gather(
                    g[:],
                    proj_d[sbk * BS:(sbk + 1) * BS, :],
                    si[:],
                    size,
                    size,
                    D,
                    queue_num=0,
                )
                nc.gpsimd.dma_scatter_add(
                    msg_d[tbk * TBS:tbk * TBS + TBS, :],
                    g[:],
                    ti[:],
                    size,
                    size,
                    D,
                    queue_num=0,
                )

    nc.compile()
    return nc


def kernel(edge_lists, node_states, W, b):
    edge_lists = np.asarray(edge_lists)
    node_states = np.asarray(node_states, dtype=np.float32)
    W = np.asarray(W, dtype=np.float32)
    b = np.asarray(b, dtype=np.float32)

    chunks, tot, gsrc_w, gtgt_w = build_schedule(edge_lists)
    nc = build_bass(chunks, tot)

    in_maps = []
    for e in range(E):
        we_t = np.ascontiguousarray(W[e * D:(e + 1) * D, :].T)         # [k, j]
        bb = np.ascontiguousarray(
            np.broadcast_to(b[e * D:(e + 1) * D], (D, D))
        )
        in_maps.append(
            {
                "x": node_states,
                "wt": we_t,
                "bb": bb,
                "gsrc": gsrc_w[e],
                "gtgt": gtgt_w[e],
            }
        )

    global LAST
    res = run_bass_kernel_spmd(nc, in_maps, core_ids=list(range(E)), trace=TRACE)
    LAST = res

    total = np.zeros((N, D), dtype=np.float32)
    for e in range(E):
        buf = res.results[e]["msg"]
        for bkt in range(NB):
            total[bkt * BS:(bkt + 1) * BS] += buf[bkt * TBS:bkt * TBS + BS]

    counts = np.bincount(
        np.asarray(edge_lists[:, :, 1]).reshape(-1), minlength=N
    ).astype(np.float32)
    divisor = np.where(counts == 0.0, 1.0, counts)
    return total / divisor[:, None]



# revision 2
# speedup vs baseline: 1.0231x; 1.0231x over previous
"""GNN message-passing layer on 8 Trainium2 NeuronCores — gather + matmul-aggregation.

Per core e (one edge type per core):
    agg[t, :] = sum_{j: tgt_j = t} x[src_j, :]        (edges of type e)
    msgT_e    = W_e^T-applied transpose:  msgT[d2, t] = (agg @ W_e^T)^T
Host:  msg = sum_e msgT_e^T + sum_e outer(bincount(tgt_e), b_e); divide by counts.

Device pipeline (no scatter, no projection pass):
  - x fp16 in DRAM (host-cast). Edges target-sorted, grouped by
    (gather-group g of GW windows, src bucket b, window w).
  - dma_gather (GPSIMD/SWDGE) pulls x rows per (g, b) call, fp16.
  - For each (segment-run): DVE builds one-hot S [K, 512] via is_equal
    (iota row vs per-partition relative target), PE matmuls
    psum_w[d, t] += Gx[e0:e0+K, d]^T @ S[e0:e0+K, t]  (accumulate).
  - Window retire: DVE copies psum->fp16, PE applies W_e^T, scalar engine
    copies fp16, sync DMA writes msgT[:, w*512:...] sequentially.

Only the gather goes through the Q7 descriptor-generation path (the
baseline's bottleneck); per-core token count ~= 200k real + ~35k pad.
"""

import numpy as np

import concourse.bacc as bacc
import concourse.bass as bass
import concourse.mybir as mybir
import concourse.tile as tile
from concourse.bass_utils import run_bass_kernel_spmd

N = 100000   # nodes
D = 128      # hidden
E = 8        # edge types == cores
M = 200000   # edges per type

NB = 4        # src buckets (int16 gather index windows)
BS = 25000    # bucket size
WD = 512      # targets per window (psum free dim)
NWIN = (N + WD - 1) // WD        # 196
GW = 8        # windows per gather group
NG = (NWIN + GW - 1) // GW       # 25 groups
DMA_SCRATCH = 16384              # SWDGE ring (default)
MAX_CALL = 1024                  # per-gather-call index cap (ucode/ring bound)
CAP_AL = 64   # cap alignment (PE tile-position grid)
SENT = 600.0  # pad sentinel for relative targets (outside [0, 512))

F32 = mybir.dt.float32
F16 = mybir.dt.float16
I16 = mybir.dt.int16

TRACE = False
LAST = None


def build_schedule(edge_lists):
    """Common (cross-core) schedule + per-core index/target data.

    Returns dict with:
      caps      [NWIN, NB] int          harmonized token counts (%64)
      calls     list of (g, b, off, n)  gather calls (token offsets, %16)
      runs      list of (call_idx, stripe, p0, K, w, trel_col, first, last)
      tot       total tokens
      nruns     number of runs
      gidx      [E, 128, tot//16] int16 wrapped gather indices
      trel      [E, 128, nruns] float32 relative targets (SENT for pads)
      counts_e  [E, N] int64            per-type target bincounts
    """
    assert edge_lists.shape == (E, M, 2)
    src = np.asarray(edge_lists[:, :, 0], dtype=np.int64)
    tgt = np.asarray(edge_lists[:, :, 1], dtype=np.int64)
    w_of = tgt // WD
    b_of = src // BS

    # harmonized caps
    counts = np.zeros((E, NWIN, NB), dtype=np.int64)
    for e in range(E):
        np.add.at(counts[e], (w_of[e], b_of[e]), 1)
    caps = -(-counts.max(axis=0) // CAP_AL) * CAP_AL          # [NWIN, NB]

    # gather calls: one per (group, bucket); segment (w, b) tokens live at
    # call-local offsets in window order.
    # gather calls, split at MAX_CALL tokens (SWDGE per-op descriptor
    # bound).  Segment (w, b) tokens live at call-local offsets in window
    # order; each call is bucket-pure.
    assert MAX_CALL % 128 == 0
    calls = []            # (g, b, global_off, n_tokens)
    seg_gbase = {}        # (w, b) -> global token offset of segment
    off = 0
    for g in range(NG):
        wlo, whi = g * GW, min((g + 1) * GW, NWIN)
        for b in range(NB):
            n = int(caps[wlo:whi, b].sum())
            loc = 0
            for w in range(wlo, whi):
                seg_gbase[(w, b)] = off + loc
                loc += int(caps[w, b])
            npad = -(-n // 128) * 128   # full stripes: no stale SBUF reads
            so = 0
            while so < npad:
                sn = min(MAX_CALL, npad - so)
                calls.append((g, b, off + so, sn))
                so += sn
            off += npad
    tot = off
    assert tot % 128 == 0

    # map a global token position to (call_idx, stripe, within-stripe)
    call_offs = np.array([c[2] for c in calls])

    def pos_to_call(gpos):
        ci = int(np.searchsorted(call_offs, gpos, side="right")) - 1
        local = gpos - calls[ci][2]
        return ci, local // 128, local % 128

    # runs: per (w, b) segment, split at 128-stripe boundaries of its call's
    # local position space.  p0 in {0, 64} guaranteed by CAP_AL=64 and
    # MAX_CALL % 128 == 0.
    runs = []
    for g in range(NG):
        wlo, whi = g * GW, min((g + 1) * GW, NWIN)
        for b in range(NB):
            for w in range(wlo, whi):
                cap = int(caps[w, b])
                if cap == 0:
                    continue
                q = seg_gbase[(w, b)]
                rem = cap
                while rem > 0:
                    ci, stripe, p0 = pos_to_call(q)
                    k = min(rem, 128 - p0, calls[ci][2] + calls[ci][3] - q)
                    assert p0 in (0, 64) and (p0 == 0 or k <= 64), (p0, k)
                    runs.append([ci, stripe, p0, k, w, len(runs),
                                 False, False, q])
                    q += k
                    rem -= k

    # mark first/last run per window (for psum start flag / stop flag)
    seen_first = set()
    last_by_w = {}
    for r in runs:
        wv = r[4]
        if wv not in seen_first:
            r[6] = True
            seen_first.add(wv)
        last_by_w[wv] = r
    for r in last_by_w.values():
        r[7] = True
    nruns = len(runs)

    # per-core data
    gidx = np.zeros((E, tot), dtype=np.int16)
    trel = np.full((E, 128, nruns), SENT, dtype=np.float32)
    for e in range(E):
        order = np.lexsort((tgt[e], b_of[e], w_of[e]))
        s_srt = src[e][order]
        t_srt = tgt[e][order]
        w_srt = w_of[e][order]
        b_srt = b_of[e][order]
        # boundaries of (w, b) groups in sorted order
        key = w_srt * NB + b_srt
        starts = np.flatnonzero(np.r_[True, key[1:] != key[:-1]])
        ends = np.r_[starts[1:], len(key)]
        seg_start_sorted = {}
        for s0, s1 in zip(starts, ends):
            w = int(w_srt[s0])
            b = int(b_srt[s0])
            n = s1 - s0
            base = seg_gbase[(w, b)]
            gidx[e, base:base + n] = (s_srt[s0:s1] % BS).astype(np.int16)
            seg_start_sorted[(w, b)] = (s0, s1)
        # fill trel per run
        for r in runs:
            ci, stripe, p0, k, w, col, _, _, g0 = r
            b = calls[ci][1]
            ss = seg_start_sorted.get((w, b))
            if ss is None:
                continue
            s0, s1 = ss
            nreal = s1 - s0
            lo = g0 - seg_gbase[(w, b)]         # run offset within segment
            hi = min(lo + k, nreal)
            if hi > lo:
                rel = (t_srt[s0 + lo:s0 + hi] - w * WD).astype(np.float32)
                trel[e, p0:p0 + (hi - lo), col] = rel

    # wrap gidx [tot] -> [128, tot//16] (token j at (j%16, j//16), replicated)
    gidx_w = np.tile(gidx.reshape(E, -1, 16).transpose(0, 2, 1), (1, 8, 1))

    counts_e = np.zeros((E, N), dtype=np.int64)
    for e in range(E):
        counts_e[e] = np.bincount(tgt[e], minlength=N)

    return dict(
        caps=caps, calls=calls, runs=runs, tot=tot, nruns=nruns,
        gidx=np.ascontiguousarray(gidx_w), trel=trel, counts_e=counts_e,
    )


def build_bass(sched):
    calls = sched["calls"]
    runs = sched["runs"]
    tot = sched["tot"]
    nruns = sched["nruns"]

    for (_, _, _, n) in calls:
        assert n <= MAX_CALL, f"gather call of {n} idx exceeds ring bound"

    nc = bacc.Bacc("TRN2", target_bir_lowering=False,
                   dynamic_dma_scratch_size=DMA_SCRATCH)
    x_d = nc.dram_tensor("x", [N, D], F16, kind="ExternalInput")
    wt_d = nc.dram_tensor("wt", [D, D], F16, kind="ExternalInput")   # W_e^T
    gi_d = nc.dram_tensor("gidx", [128, tot // 16], I16, kind="ExternalInput")
    tr_d = nc.dram_tensor("trel", [128, nruns], F32, kind="ExternalInput")
    io_d = nc.dram_tensor("iota", [128, WD], F16, kind="ExternalInput")
    out_d = nc.dram_tensor("msgT", [128, N], F16, kind="ExternalOutput")

    # group runs by window (emission order) and by call (for gather emission)
    runs_by_w = {}
    for r in runs:
        runs_by_w.setdefault(r[4], []).append(r)
    first_use_group = {}  # call_idx -> first group in which used == its own g
    # calls needed for window w: those of group w//GW

    with tile.TileContext(nc) as tc:
        with (
            tc.tile_pool(name="const", bufs=1) as constp,
            tc.tile_pool(name="gx", bufs=24) as gxp,
            tc.tile_pool(name="s", bufs=4) as sp,
            tc.tile_pool(name="aggps", bufs=4, space="PSUM") as aggp,
            tc.tile_pool(name="wps", bufs=2, space="PSUM") as wpsp,
            tc.tile_pool(name="aggs", bufs=3) as aggsp,
            tc.tile_pool(name="outp", bufs=3) as outp,
        ):
            wt_s = constp.tile([D, D], F16)
            nc.sync.dma_start(wt_s[:], wt_d[:])
            iota_s = constp.tile([128, WD], F16)
            nc.sync.dma_start(iota_s[:], io_d[:])
            trel_s = constp.tile([128, nruns], F32)
            nc.sync.dma_start(trel_s[:], tr_d[:])
            gi_s = constp.tile([128, tot // 16], I16)
            nc.sync.dma_start(gi_s[:], gi_d[:])

            gx_tiles = {}

            def emit_gathers(g):
                for ci, (gg, b, off, n) in enumerate(calls):
                    if gg != g or n == 0:
                        continue
                    nst = -(-n // 128)
                    gxt = gxp.tile([128, nst, D], F16, tag="gx",
                                   name=f"gx{ci}")
                    nc.gpsimd.dma_gather(
                        gxt[:], x_d[b * BS:(b + 1) * BS, :],
                        gi_s[:, off // 16:(off + n) // 16],
                        n, n, D, queue_num=0,
                    )
                    gx_tiles[ci] = gxt

            emit_gathers(0)
            emit_gathers(1)

            retire_q = []

            def retire(w, ps):
                nwd = min(WD, N - w * WD)
                a_s = aggsp.tile([128, WD], F16, tag="aggs", name=f"aggs{w}")
                nc.scalar.copy(a_s[:], ps[:])
                wps = wpsp.tile([128, WD], F32, tag="wps", name=f"wps{w}")
                nc.tensor.matmul(wps[:], wt_s[:], a_s[:],
                                 start=True, stop=True, skip_group_check=True)
                o_s = outp.tile([128, WD], F16, tag="out", name=f"out{w}")
                nc.scalar.copy(o_s[:], wps[:])
                nc.sync.dma_start(out_d[:, w * WD:w * WD + nwd], o_s[:, :nwd])

            for w in range(NWIN):
                if w % GW == 0 and w // GW + 2 <= NG - 1:
                    emit_gathers(w // GW + 2)
                ps = aggp.tile([128, WD], F32, tag="agg", name=f"agg{w}")
                for r in runs_by_w.get(w, []):
                    ci, stripe, p0, k, _, col, first, last = r[:8]
                    gxt = gx_tiles[ci]
                    # full 128-partition S build: partitions outside the run
                    # hold SENT in trel -> all-zero rows -> no contribution.
                    # (PE tile_position != 0 is broken at scale on HW; keep
                    # every matmul K=128 at partition 0.)
                    s_t = sp.tile([128, WD], F16, tag="s", name=f"s{col}")
                    nc.vector.tensor_scalar(
                        s_t[:], iota_s[:],
                        trel_s[:, col:col + 1], None,
                        op0=mybir.AluOpType.is_equal,
                    )
                    nc.tensor.matmul(
                        ps[:], gxt[:, stripe, :], s_t[:],
                        start=first, stop=last, skip_group_check=True,
                    )
                retire_q.append((w, ps))
                if len(retire_q) > 1:
                    retire(*retire_q.pop(0))
            while retire_q:
                retire(*retire_q.pop(0))

    nc.compile()
    return nc


def kernel(edge_lists, node_states, W, b):
    edge_lists = np.asarray(edge_lists)
    node_states = np.asarray(node_states, dtype=np.float32)
    W = np.asarray(W, dtype=np.float32)
    b = np.asarray(b, dtype=np.float32)

    sched = build_schedule(edge_lists)
    nc = build_bass(sched)

    x16 = node_states.astype(np.float16)
    iota = np.tile(np.arange(WD, dtype=np.float16), (128, 1))
    in_maps = []
    for e in range(E):
        wt16 = np.ascontiguousarray(W[e * D:(e + 1) * D, :].T).astype(np.float16)
        in_maps.append({
            "x": x16,
            "wt": wt16,
            "gidx": sched["gidx"][e],
            "trel": sched["trel"][e],
            "iota": iota,
        })

    global LAST
    res = run_bass_kernel_spmd(nc, in_maps, core_ids=list(range(E)), trace=TRACE)
    LAST = res

    total = np.zeros((N, D), dtype=np.float32)
    for e in range(E):
        total += res.results[e]["msgT"].astype(np.float32).T
    counts_e = sched["counts_e"].astype(np.float32)
    for e in range(E):
        total += np.outer(counts_e[e], b[e * D:(e + 1) * D])
    counts = counts_e.sum(axis=0)
    divisor = np.where(counts == 0.0, 1.0, counts)
    return (total / divisor[:, None]).astype(np.float32)


def selfcheck_schedule(edge_lists, node_states, W, b):
    """Numpy emulation of the device program for schedule validation."""
    sched = build_schedule(np.asarray(edge_lists))
    x16 = np.asarray(node_states, dtype=np.float32).astype(np.float16)
    calls, runs = sched["calls"], sched["runs"]
    total = np.zeros((N, D), dtype=np.float32)
    for e in range(E):
        # emulate gather
        gidx_w = sched["gidx"][e]
        gvals = {}
        for ci, (g, bkt, off, n) in enumerate(calls):
            if n == 0:
                continue
            cols = gidx_w[:16, off // 16:(off + n) // 16]
            idxs = cols.T.reshape(-1)[:n].astype(np.int64)
            rows = x16[bkt * BS + idxs]          # [n, D]
            nst = -(-n // 128)
            buf = np.zeros((128, nst, D), np.float16)
            pos = np.arange(n)
            buf[pos % 128, pos // 128] = rows
            gvals[ci] = buf
        # emulate windows
        msgT = np.zeros((128, N), dtype=np.float32)
        wt16 = np.ascontiguousarray(W[e * D:(e + 1) * D, :].T).astype(np.float16)
        psums = {}
        for r in runs:
            ci, stripe, p0, k, w, col, first, last = r[:8]
            if first:
                psums[w] = np.zeros((128, WD), np.float32)
            gx = gvals[ci][:, stripe, :].astype(np.float32)   # [128, D]
            rel = sched["trel"][e][:, col]                    # [128]
            S = (rel[:, None] == np.arange(WD)[None, :]).astype(np.float32)
            psums[w] += gx.T @ S
        for w, ps in psums.items():
            nwd = min(WD, N - w * WD)
            agg16 = ps.astype(np.float16).astype(np.float32)
            m = (wt16.astype(np.float32).T @ agg16).astype(np.float16)
            msgT[:, w * WD:w * WD + nwd] = m[:, :nwd].astype(np.float32)
        total += msgT.T
    counts_e = sched["counts_e"].astype(np.float32)
    bb = np.asarray(b, dtype=np.float32)
    for e in range(E):
        total += np.outer(counts_e[e], bb[e * D:(e + 1) * D])
    counts = counts_e.sum(axis=0)
    divisor = np.where(counts == 0.0, 1.0, counts)
    return (total / divisor[:, None]).astype(np.float32)


# revision 3
# speedup vs baseline: 1.1177x; 1.0925x over previous
"""GNN message-passing layer on 8 Trainium2 NeuronCores — gather + matmul-aggregation.

Per core e (one edge type per core):
    agg[t, :] = sum_{j: tgt_j = t} x[src_j, :]        (edges of type e)
    msgT_e    = W_e^T-applied transpose:  msgT[d2, t] = (agg @ W_e^T)^T
Host:  msg = sum_e msgT_e^T + sum_e outer(bincount(tgt_e), b_e); divide by counts.

Device pipeline (no scatter, no projection pass):
  - x fp16 in DRAM (host-cast). Edges target-sorted, grouped by
    (gather-group g of GW windows, src bucket b, window w).
  - dma_gather (GPSIMD/SWDGE) pulls x rows per (g, b) call, fp16.
  - For each (segment-run): DVE builds one-hot S [K, 512] via is_equal
    (iota row vs per-partition relative target), PE matmuls
    psum_w[d, t] += Gx[e0:e0+K, d]^T @ S[e0:e0+K, t]  (accumulate).
  - Window retire: DVE copies psum->fp16, PE applies W_e^T, scalar engine
    copies fp16, sync DMA writes msgT[:, w*512:...] sequentially.

Only the gather goes through the Q7 descriptor-generation path (the
baseline's bottleneck); per-core token count ~= 200k real + ~35k pad.
"""

import numpy as np

import concourse.bacc as bacc
import concourse.bass as bass
import concourse.mybir as mybir
import concourse.tile as tile
from concourse.bass_utils import run_bass_kernel_spmd

N = 100000   # nodes
D = 128      # hidden
E = 8        # edge types == cores
M = 200000   # edges per type

NB = 4        # src buckets (int16 gather index windows)
BS = 25000    # bucket size
WD = 512      # targets per window (psum free dim)
NWIN = (N + WD - 1) // WD        # 196
GW = 8        # windows per gather group
NG = (NWIN + GW - 1) // GW       # 25 groups
DMA_SCRATCH = 16384              # SWDGE ring (default)
MAX_CALL = 1024                  # per-gather-call index cap (ucode/ring bound)
CAP_AL = 64   # cap alignment (PE tile-position grid)
SENT = 600.0  # pad sentinel for relative targets (outside [0, 512))

F32 = mybir.dt.float32
F16 = mybir.dt.float16
I16 = mybir.dt.int16

TRACE = False
LAST = None


def build_schedule(edge_lists):
    """Common (cross-core) schedule + per-core index/target data.

    Returns dict with:
      caps      [NWIN, NB] int          harmonized token counts (%64)
      calls     list of (g, b, off, n)  gather calls (token offsets, %16)
      runs      list of (call_idx, stripe, p0, K, w, trel_col, first, last)
      tot       total tokens
      nruns     number of runs
      gidx      [E, 128, tot//16] int16 wrapped gather indices
      trel      [E, 128, nruns] float32 relative targets (SENT for pads)
      counts_e  [E, N] int64            per-type target bincounts
    """
    assert edge_lists.shape == (E, M, 2)
    src = np.asarray(edge_lists[:, :, 0], dtype=np.int64)
    tgt = np.asarray(edge_lists[:, :, 1], dtype=np.int64)
    w_of = tgt // WD
    b_of = src // BS

    # harmonized caps
    counts = np.zeros((E, NWIN, NB), dtype=np.int64)
    for e in range(E):
        np.add.at(counts[e], (w_of[e], b_of[e]), 1)
    caps = -(-counts.max(axis=0) // CAP_AL) * CAP_AL          # [NWIN, NB]

    # gather calls: one per (group, bucket); segment (w, b) tokens live at
    # call-local offsets in window order.
    # gather calls, split at MAX_CALL tokens (SWDGE per-op descriptor
    # bound).  Segment (w, b) tokens live at call-local offsets in window
    # order; each call is bucket-pure.
    assert MAX_CALL % 128 == 0
    calls = []            # (g, b, global_off, n_tokens)
    seg_gbase = {}        # (w, b) -> global token offset of segment
    off = 0
    for g in range(NG):
        wlo, whi = g * GW, min((g + 1) * GW, NWIN)
        for b in range(NB):
            n = int(caps[wlo:whi, b].sum())
            loc = 0
            for w in range(wlo, whi):
                seg_gbase[(w, b)] = off + loc
                loc += int(caps[w, b])
            npad = -(-n // 128) * 128   # full stripes: no stale SBUF reads
            so = 0
            while so < npad:
                sn = min(MAX_CALL, npad - so)
                calls.append((g, b, off + so, sn))
                so += sn
            off += npad
    tot = off
    assert tot % 128 == 0

    # map a global token position to (call_idx, stripe, within-stripe)
    call_offs = np.array([c[2] for c in calls])

    def pos_to_call(gpos):
        ci = int(np.searchsorted(call_offs, gpos, side="right")) - 1
        local = gpos - calls[ci][2]
        return ci, local // 128, local % 128

    # runs: per (w, b) segment, split at 128-stripe boundaries of its call's
    # local position space.  p0 in {0, 64} guaranteed by CAP_AL=64 and
    # MAX_CALL % 128 == 0.
    runs = []
    for g in range(NG):
        wlo, whi = g * GW, min((g + 1) * GW, NWIN)
        for b in range(NB):
            for w in range(wlo, whi):
                cap = int(caps[w, b])
                if cap == 0:
                    continue
                q = seg_gbase[(w, b)]
                rem = cap
                while rem > 0:
                    ci, stripe, p0 = pos_to_call(q)
                    k = min(rem, 128 - p0, calls[ci][2] + calls[ci][3] - q)
                    assert p0 in (0, 64) and (p0 == 0 or k <= 64), (p0, k)
                    runs.append([ci, stripe, p0, k, w, len(runs),
                                 False, False, q])
                    q += k
                    rem -= k

    # mark first/last run per window (for psum start flag / stop flag)
    seen_first = set()
    last_by_w = {}
    for r in runs:
        wv = r[4]
        if wv not in seen_first:
            r[6] = True
            seen_first.add(wv)
        last_by_w[wv] = r
    for r in last_by_w.values():
        r[7] = True
    nruns = len(runs)

    # per-core data
    gidx = np.zeros((E, tot), dtype=np.int16)
    trel = np.full((E, 128, nruns), SENT, dtype=np.float32)
    for e in range(E):
        order = np.lexsort((tgt[e], b_of[e], w_of[e]))
        s_srt = src[e][order]
        t_srt = tgt[e][order]
        w_srt = w_of[e][order]
        b_srt = b_of[e][order]
        # boundaries of (w, b) groups in sorted order
        key = w_srt * NB + b_srt
        starts = np.flatnonzero(np.r_[True, key[1:] != key[:-1]])
        ends = np.r_[starts[1:], len(key)]
        seg_start_sorted = {}
        for s0, s1 in zip(starts, ends):
            w = int(w_srt[s0])
            b = int(b_srt[s0])
            n = s1 - s0
            base = seg_gbase[(w, b)]
            gidx[e, base:base + n] = (s_srt[s0:s1] % BS).astype(np.int16)
            seg_start_sorted[(w, b)] = (s0, s1)
        # fill trel per run
        for r in runs:
            ci, stripe, p0, k, w, col, _, _, g0 = r
            b = calls[ci][1]
            ss = seg_start_sorted.get((w, b))
            if ss is None:
                continue
            s0, s1 = ss
            nreal = s1 - s0
            lo = g0 - seg_gbase[(w, b)]         # run offset within segment
            hi = min(lo + k, nreal)
            if hi > lo:
                rel = (t_srt[s0 + lo:s0 + hi] - w * WD).astype(np.float32)
                trel[e, p0:p0 + (hi - lo), col] = rel

    # per-run union target spans across cores (columns actually non-zero
    # in S); first run of each window stays full-width so its start=True
    # matmul initializes the whole psum bank.
    spans = []
    for r in runs:
        col = r[5]
        vals = trel[:, :, col]
        real = vals < SENT
        if r[6] or not real.any():
            spans.append((0, WD))
            continue
        c0 = int(vals[real].min()) & ~1
        c1 = min(WD, (int(vals[real].max()) + 2) & ~1)
        spans.append((c0, c1))

    # wrap gidx [tot] -> [128, tot//16] (token j at (j%16, j//16), replicated)
    gidx_w = np.tile(gidx.reshape(E, -1, 16).transpose(0, 2, 1), (1, 8, 1))

    counts_e = np.zeros((E, N), dtype=np.int64)
    for e in range(E):
        counts_e[e] = np.bincount(tgt[e], minlength=N)

    return dict(
        caps=caps, calls=calls, runs=runs, tot=tot, nruns=nruns, spans=spans,
        gidx=np.ascontiguousarray(gidx_w), trel=trel, counts_e=counts_e,
    )


def build_bass(sched):
    calls = sched["calls"]
    runs = sched["runs"]
    tot = sched["tot"]
    nruns = sched["nruns"]
    spans = sched["spans"]

    for (_, _, _, n) in calls:
        assert n <= MAX_CALL, f"gather call of {n} idx exceeds ring bound"

    nc = bacc.Bacc("TRN2", target_bir_lowering=False,
                   dynamic_dma_scratch_size=DMA_SCRATCH)
    x_d = nc.dram_tensor("x", [N, D], F16, kind="ExternalInput")
    wt_d = nc.dram_tensor("wt", [D, D], F16, kind="ExternalInput")   # W_e^T
    gi_d = nc.dram_tensor("gidx", [128, tot // 16], I16, kind="ExternalInput")
    tr_d = nc.dram_tensor("trel", [128, nruns], F32, kind="ExternalInput")
    io_d = nc.dram_tensor("iota", [128, WD], F16, kind="ExternalInput")
    out_d = nc.dram_tensor("msgT", [128, N], F16, kind="ExternalOutput")

    # group runs by window (emission order) and by call (for gather emission)
    runs_by_w = {}
    for r in runs:
        runs_by_w.setdefault(r[4], []).append(r)
    first_use_group = {}  # call_idx -> first group in which used == its own g
    # calls needed for window w: those of group w//GW

    with tile.TileContext(nc) as tc:
        with (
            tc.tile_pool(name="const", bufs=1) as constp,
            tc.tile_pool(name="gx", bufs=24) as gxp,
            tc.tile_pool(name="s", bufs=4) as sp,
            tc.tile_pool(name="aggps", bufs=4, space="PSUM") as aggp,
            tc.tile_pool(name="wps", bufs=2, space="PSUM") as wpsp,
            tc.tile_pool(name="aggs", bufs=3) as aggsp,
            tc.tile_pool(name="outp", bufs=3) as outp,
        ):
            wt_s = constp.tile([D, D], F16)
            nc.sync.dma_start(wt_s[:], wt_d[:])
            iota_s = constp.tile([128, WD], F16)
            nc.sync.dma_start(iota_s[:], io_d[:])
            trel_s = constp.tile([128, nruns], F32)
            nc.sync.dma_start(trel_s[:], tr_d[:])
            gi_s = constp.tile([128, tot // 16], I16)
            nc.sync.dma_start(gi_s[:], gi_d[:])

            gx_tiles = {}

            def emit_gathers(g):
                for ci, (gg, b, off, n) in enumerate(calls):
                    if gg != g or n == 0:
                        continue
                    nst = -(-n // 128)
                    gxt = gxp.tile([128, nst, D], F16, tag="gx",
                                   name=f"gx{ci}")
                    nc.gpsimd.dma_gather(
                        gxt[:], x_d[b * BS:(b + 1) * BS, :],
                        gi_s[:, off // 16:(off + n) // 16],
                        n, n, D, queue_num=0,
                    )
                    gx_tiles[ci] = gxt

            emit_gathers(0)
            emit_gathers(1)

            retire_q = []

            def retire(w, ps):
                nwd = min(WD, N - w * WD)
                a_s = aggsp.tile([128, WD], F16, tag="aggs", name=f"aggs{w}")
                nc.scalar.copy(a_s[:], ps[:])
                wps = wpsp.tile([128, WD], F32, tag="wps", name=f"wps{w}")
                nc.tensor.matmul(wps[:], wt_s[:], a_s[:],
                                 start=True, stop=True, skip_group_check=True)
                o_s = outp.tile([128, WD], F16, tag="out", name=f"out{w}")
                nc.scalar.copy(o_s[:], wps[:])
                nc.sync.dma_start(out_d[:, w * WD:w * WD + nwd], o_s[:, :nwd])

            for w in range(NWIN):
                if w % GW == 0 and w // GW + 2 <= NG - 1:
                    emit_gathers(w // GW + 2)
                ps = aggp.tile([128, WD], F32, tag="agg", name=f"agg{w}")
                for r in runs_by_w.get(w, []):
                    ci, stripe, p0, k, _, col, first, last = r[:8]
                    gxt = gx_tiles[ci]
                    # full 128-partition S build: partitions outside the run
                    # hold SENT in trel -> all-zero rows -> no contribution.
                    # (PE tile_position != 0 is broken at scale on HW; keep
                    # every matmul K=128 at partition 0.)
                    c0, c1 = spans[col]
                    wc = c1 - c0
                    s_t = sp.tile([128, WD], F16, tag="s", name=f"s{col}")
                    nc.vector.tensor_scalar(
                        s_t[:, 0:wc], iota_s[:, c0:c1],
                        trel_s[:, col:col + 1], None,
                        op0=mybir.AluOpType.is_equal,
                    )
                    nc.tensor.matmul(
                        ps[:, c0:c1], gxt[:, stripe, :], s_t[:, 0:wc],
                        start=first, stop=last, skip_group_check=True,
                    )
                retire_q.append((w, ps))
                if len(retire_q) > 1:
                    retire(*retire_q.pop(0))
            while retire_q:
                retire(*retire_q.pop(0))

    nc.compile()
    return nc


def kernel(edge_lists, node_states, W, b):
    edge_lists = np.asarray(edge_lists)
    node_states = np.asarray(node_states, dtype=np.float32)
    W = np.asarray(W, dtype=np.float32)
    b = np.asarray(b, dtype=np.float32)

    sched = build_schedule(edge_lists)
    nc = build_bass(sched)

    x16 = node_states.astype(np.float16)
    iota = np.tile(np.arange(WD, dtype=np.float16), (128, 1))
    in_maps = []
    for e in range(E):
        wt16 = np.ascontiguousarray(W[e * D:(e + 1) * D, :].T).astype(np.float16)
        in_maps.append({
            "x": x16,
            "wt": wt16,
            "gidx": sched["gidx"][e],
            "trel": sched["trel"][e],
            "iota": iota,
        })

    global LAST
    res = run_bass_kernel_spmd(nc, in_maps, core_ids=list(range(E)), trace=TRACE)
    LAST = res

    total = np.zeros((N, D), dtype=np.float32)
    for e in range(E):
        total += res.results[e]["msgT"].astype(np.float32).T
    counts_e = sched["counts_e"].astype(np.float32)
    for e in range(E):
        total += np.outer(counts_e[e], b[e * D:(e + 1) * D])
    counts = counts_e.sum(axis=0)
    divisor = np.where(counts == 0.0, 1.0, counts)
    return (total / divisor[:, None]).astype(np.float32)


def selfcheck_schedule(edge_lists, node_states, W, b):
    """Numpy emulation of the device program for schedule validation."""
    sched = build_schedule(np.asarray(edge_lists))
    x16 = np.asarray(node_states, dtype=np.float32).astype(np.float16)
    calls, runs = sched["calls"], sched["runs"]
    total = np.zeros((N, D), dtype=np.float32)
    for e in range(E):
        # emulate gather
        gidx_w = sched["gidx"][e]
        gvals = {}
        for ci, (g, bkt, off, n) in enumerate(calls):
            if n == 0:
                continue
            cols = gidx_w[:16, off // 16:(off + n) // 16]
            idxs = cols.T.reshape(-1)[:n].astype(np.int64)
            rows = x16[bkt * BS + idxs]          # [n, D]
            nst = -(-n // 128)
            buf = np.zeros((128, nst, D), np.float16)
            pos = np.arange(n)
            buf[pos % 128, pos // 128] = rows
            gvals[ci] = buf
        # emulate windows
        msgT = np.zeros((128, N), dtype=np.float32)
        wt16 = np.ascontiguousarray(W[e * D:(e + 1) * D, :].T).astype(np.float16)
        psums = {}
        for r in runs:
            ci, stripe, p0, k, w, col, first, last = r[:8]
            if first:
                psums[w] = np.zeros((128, WD), np.float32)
            gx = gvals[ci][:, stripe, :].astype(np.float32)   # [128, D]
            rel = sched["trel"][e][:, col]                    # [128]
            S = (rel[:, None] == np.arange(WD)[None, :]).astype(np.float32)
            psums[w] += gx.T @ S
        for w, ps in psums.items():
            nwd = min(WD, N - w * WD)
            agg16 = ps.astype(np.float16).astype(np.float32)
            m = (wt16.astype(np.float32).T @ agg16).astype(np.float16)
            msgT[:, w * WD:w * WD + nwd] = m[:, :nwd].astype(np.float32)
        total += msgT.T
    counts_e = sched["counts_e"].astype(np.float32)
    bb = np.asarray(b, dtype=np.float32)
    for e in range(E):
        total += np.outer(counts_e[e], bb[e * D:(e + 1) * D])
    counts = counts_e.sum(axis=0)
    divisor = np.where(counts == 0.0, 1.0, counts)
    return (total / divisor[:, None]).astype(np.float32)


# revision 4
# speedup vs baseline: 1.1315x; 1.0123x over previous
"""GNN message-passing layer on 8 Trainium2 NeuronCores — gather + matmul-aggregation.

Per core e (one edge type per core):
    agg[t, :] = sum_{j: tgt_j = t} x[src_j, :]        (edges of type e)
    msgT_e    = W_e^T-applied transpose:  msgT[d2, t] = (agg @ W_e^T)^T
Host:  msg = sum_e msgT_e^T + sum_e outer(bincount(tgt_e), b_e); divide by counts.

Device pipeline (no scatter, no projection pass):
  - x fp16 in DRAM (host-cast). Edges target-sorted, grouped by
    (gather-group g of GW windows, src bucket b, window w).
  - dma_gather (GPSIMD/SWDGE) pulls x rows per (g, b) call, fp16.
  - For each (segment-run): DVE builds one-hot S [K, 512] via is_equal
    (iota row vs per-partition relative target), PE matmuls
    psum_w[d, t] += Gx[e0:e0+K, d]^T @ S[e0:e0+K, t]  (accumulate).
  - Window retire: DVE copies psum->fp16, PE applies W_e^T, scalar engine
    copies fp16, sync DMA writes msgT[:, w*512:...] sequentially.

Only the gather goes through the Q7 descriptor-generation path (the
baseline's bottleneck); per-core token count ~= 200k real + ~35k pad.
"""

import numpy as np

import concourse.bacc as bacc
import concourse.bass as bass
import concourse.mybir as mybir
import concourse.tile as tile
from concourse.bass_utils import run_bass_kernel_spmd

N = 100000   # nodes
D = 128      # hidden
E = 8        # edge types == cores
M = 200000   # edges per type

NB = 4        # src buckets (int16 gather index windows)
BS = 25000    # bucket size
WD = 512      # targets per window (psum free dim)
NWIN = (N + WD - 1) // WD        # 196
GW = 8        # windows per gather group
NG = (NWIN + GW - 1) // GW       # 25 groups
DMA_SCRATCH = 16384              # SWDGE ring (default)
MAX_CALL = 1024                  # per-gather-call index cap (ucode/ring bound)
CAP_AL = 16   # cap alignment (matmuls are full-stripe K=128, so any %16 works)
SENT = 600.0  # pad sentinel for relative targets (outside [0, 512))

F32 = mybir.dt.float32
F16 = mybir.dt.float16
I16 = mybir.dt.int16

TRACE = False
LAST = None


def build_schedule(edge_lists):
    """Common (cross-core) schedule + per-core index/target data.

    Returns dict with:
      caps      [NWIN, NB] int          harmonized token counts (%64)
      calls     list of (g, b, off, n)  gather calls (token offsets, %16)
      runs      list of (call_idx, stripe, p0, K, w, trel_col, first, last)
      tot       total tokens
      nruns     number of runs
      gidx      [E, 128, tot//16] int16 wrapped gather indices
      trel      [E, 128, nruns] float32 relative targets (SENT for pads)
      counts_e  [E, N] int64            per-type target bincounts
    """
    assert edge_lists.shape == (E, M, 2)
    src = np.asarray(edge_lists[:, :, 0], dtype=np.int64)
    tgt = np.asarray(edge_lists[:, :, 1], dtype=np.int64)
    w_of = tgt // WD
    b_of = src // BS

    # harmonized caps
    counts = np.zeros((E, NWIN, NB), dtype=np.int64)
    for e in range(E):
        np.add.at(counts[e], (w_of[e], b_of[e]), 1)
    caps = -(-counts.max(axis=0) // CAP_AL) * CAP_AL          # [NWIN, NB]

    # gather calls: one per (group, bucket); segment (w, b) tokens live at
    # call-local offsets in window order.
    # gather calls, split at MAX_CALL tokens (SWDGE per-op descriptor
    # bound).  Segment (w, b) tokens live at call-local offsets in window
    # order; each call is bucket-pure.
    assert MAX_CALL % 128 == 0
    calls = []            # (g, b, global_off, n_tokens)
    seg_gbase = {}        # (w, b) -> global token offset of segment
    off = 0
    for g in range(NG):
        wlo, whi = g * GW, min((g + 1) * GW, NWIN)
        for b in range(NB):
            n = int(caps[wlo:whi, b].sum())
            loc = 0
            for w in range(wlo, whi):
                seg_gbase[(w, b)] = off + loc
                loc += int(caps[w, b])
            npad = -(-n // 128) * 128   # full stripes: no stale SBUF reads
            so = 0
            while so < npad:
                sn = min(MAX_CALL, npad - so)
                calls.append((g, b, off + so, sn))
                so += sn
            off += npad
    tot = off
    assert tot % 128 == 0

    # map a global token position to (call_idx, stripe, within-stripe)
    call_offs = np.array([c[2] for c in calls])

    def pos_to_call(gpos):
        ci = int(np.searchsorted(call_offs, gpos, side="right")) - 1
        local = gpos - calls[ci][2]
        return ci, local // 128, local % 128

    # runs: per (w, b) segment, split at 128-stripe boundaries of its call's
    # local position space.  p0 in {0, 64} guaranteed by CAP_AL=64 and
    # MAX_CALL % 128 == 0.
    runs = []
    for g in range(NG):
        wlo, whi = g * GW, min((g + 1) * GW, NWIN)
        for b in range(NB):
            for w in range(wlo, whi):
                cap = int(caps[w, b])
                if cap == 0:
                    continue
                q = seg_gbase[(w, b)]
                rem = cap
                while rem > 0:
                    ci, stripe, p0 = pos_to_call(q)
                    k = min(rem, 128 - p0, calls[ci][2] + calls[ci][3] - q)
                    assert p0 % 16 == 0, p0
                    runs.append([ci, stripe, p0, k, w, len(runs),
                                 False, False, q])
                    q += k
                    rem -= k

    # mark first/last run per window (for psum start flag / stop flag)
    seen_first = set()
    last_by_w = {}
    for r in runs:
        wv = r[4]
        if wv not in seen_first:
            r[6] = True
            seen_first.add(wv)
        last_by_w[wv] = r
    for r in last_by_w.values():
        r[7] = True
    nruns = len(runs)

    # per-core data
    gidx = np.zeros((E, tot), dtype=np.int16)
    trel = np.full((E, 128, nruns), SENT, dtype=np.float32)
    for e in range(E):
        order = np.lexsort((tgt[e], b_of[e], w_of[e]))
        s_srt = src[e][order]
        t_srt = tgt[e][order]
        w_srt = w_of[e][order]
        b_srt = b_of[e][order]
        # boundaries of (w, b) groups in sorted order
        key = w_srt * NB + b_srt
        starts = np.flatnonzero(np.r_[True, key[1:] != key[:-1]])
        ends = np.r_[starts[1:], len(key)]
        seg_start_sorted = {}
        for s0, s1 in zip(starts, ends):
            w = int(w_srt[s0])
            b = int(b_srt[s0])
            n = s1 - s0
            base = seg_gbase[(w, b)]
            gidx[e, base:base + n] = (s_srt[s0:s1] % BS).astype(np.int16)
            seg_start_sorted[(w, b)] = (s0, s1)
        # fill trel per run
        for r in runs:
            ci, stripe, p0, k, w, col, _, _, g0 = r
            b = calls[ci][1]
            ss = seg_start_sorted.get((w, b))
            if ss is None:
                continue
            s0, s1 = ss
            nreal = s1 - s0
            lo = g0 - seg_gbase[(w, b)]         # run offset within segment
            hi = min(lo + k, nreal)
            if hi > lo:
                rel = (t_srt[s0 + lo:s0 + hi] - w * WD).astype(np.float32)
                trel[e, p0:p0 + (hi - lo), col] = rel

    # per-run union target spans across cores (columns actually non-zero
    # in S); first run of each window stays full-width so its start=True
    # matmul initializes the whole psum bank.
    spans = []
    for r in runs:
        col = r[5]
        vals = trel[:, :, col]
        real = vals < SENT
        if r[6] or not real.any():
            spans.append((0, WD))
            continue
        c0 = int(vals[real].min()) & ~1
        c1 = min(WD, (int(vals[real].max()) + 2) & ~1)
        spans.append((c0, c1))

    # wrap gidx [tot] -> [128, tot//16] (token j at (j%16, j//16), replicated)
    gidx_w = np.tile(gidx.reshape(E, -1, 16).transpose(0, 2, 1), (1, 8, 1))

    counts_e = np.zeros((E, N), dtype=np.int64)
    for e in range(E):
        counts_e[e] = np.bincount(tgt[e], minlength=N)

    return dict(
        caps=caps, calls=calls, runs=runs, tot=tot, nruns=nruns, spans=spans,
        gidx=np.ascontiguousarray(gidx_w), trel=trel, counts_e=counts_e,
    )


def build_bass(sched):
    calls = sched["calls"]
    runs = sched["runs"]
    tot = sched["tot"]
    nruns = sched["nruns"]
    spans = sched["spans"]

    for (_, _, _, n) in calls:
        assert n <= MAX_CALL, f"gather call of {n} idx exceeds ring bound"

    nc = bacc.Bacc("TRN2", target_bir_lowering=False,
                   dynamic_dma_scratch_size=DMA_SCRATCH)
    x_d = nc.dram_tensor("x", [N, D], F16, kind="ExternalInput")
    wt_d = nc.dram_tensor("wt", [D, D], F16, kind="ExternalInput")   # W_e^T
    gi_d = nc.dram_tensor("gidx", [128, tot // 16], I16, kind="ExternalInput")
    tr_d = nc.dram_tensor("trel", [128, nruns], F32, kind="ExternalInput")
    io_d = nc.dram_tensor("iota", [128, WD], F16, kind="ExternalInput")
    out_d = nc.dram_tensor("msgT", [128, N], F16, kind="ExternalOutput")

    # group runs by window (emission order) and by call (for gather emission)
    runs_by_w = {}
    for r in runs:
        runs_by_w.setdefault(r[4], []).append(r)
    first_use_group = {}  # call_idx -> first group in which used == its own g
    # calls needed for window w: those of group w//GW

    with tile.TileContext(nc) as tc:
        with (
            tc.tile_pool(name="const", bufs=1) as constp,
            tc.tile_pool(name="gx", bufs=24) as gxp,
            tc.tile_pool(name="s", bufs=4) as sp,
            tc.tile_pool(name="aggps", bufs=4, space="PSUM") as aggp,
            tc.tile_pool(name="wps", bufs=2, space="PSUM") as wpsp,
            tc.tile_pool(name="aggs", bufs=3) as aggsp,
            tc.tile_pool(name="outp", bufs=3) as outp,
        ):
            wt_s = constp.tile([D, D], F16)
            nc.sync.dma_start(wt_s[:], wt_d[:])
            iota_s = constp.tile([128, WD], F16)
            nc.sync.dma_start(iota_s[:], io_d[:])
            trel_s = constp.tile([128, nruns], F32)
            nc.sync.dma_start(trel_s[:], tr_d[:])
            gi_s = constp.tile([128, tot // 16], I16)
            nc.sync.dma_start(gi_s[:], gi_d[:])

            gx_tiles = {}

            def emit_gathers(g):
                for ci, (gg, b, off, n) in enumerate(calls):
                    if gg != g or n == 0:
                        continue
                    nst = -(-n // 128)
                    gxt = gxp.tile([128, nst, D], F16, tag="gx",
                                   name=f"gx{ci}")
                    nc.gpsimd.dma_gather(
                        gxt[:], x_d[b * BS:(b + 1) * BS, :],
                        gi_s[:, off // 16:(off + n) // 16],
                        n, n, D, queue_num=0,
                    )
                    gx_tiles[ci] = gxt

            emit_gathers(0)
            emit_gathers(1)

            retire_q = []

            def retire(w, ps):
                nwd = min(WD, N - w * WD)
                a_s = aggsp.tile([128, WD], F16, tag="aggs", name=f"aggs{w}")
                nc.scalar.copy(a_s[:], ps[:])
                wps = wpsp.tile([128, WD], F32, tag="wps", name=f"wps{w}")
                nc.tensor.matmul(wps[:], wt_s[:], a_s[:],
                                 start=True, stop=True, skip_group_check=True)
                o_s = outp.tile([128, WD], F16, tag="out", name=f"out{w}")
                nc.scalar.copy(o_s[:], wps[:])
                nc.sync.dma_start(out_d[:, w * WD:w * WD + nwd], o_s[:, :nwd])

            for w in range(NWIN):
                if w % GW == 0 and w // GW + 2 <= NG - 1:
                    emit_gathers(w // GW + 2)
                ps = aggp.tile([128, WD], F32, tag="agg", name=f"agg{w}")
                for r in runs_by_w.get(w, []):
                    ci, stripe, p0, k, _, col, first, last = r[:8]
                    gxt = gx_tiles[ci]
                    # full 128-partition S build: partitions outside the run
                    # hold SENT in trel -> all-zero rows -> no contribution.
                    # (PE tile_position != 0 is broken at scale on HW; keep
                    # every matmul K=128 at partition 0.)
                    c0, c1 = spans[col]
                    wc = c1 - c0
                    s_t = sp.tile([128, WD], F16, tag="s", name=f"s{col}")
                    nc.vector.tensor_scalar(
                        s_t[:, 0:wc], iota_s[:, c0:c1],
                        trel_s[:, col:col + 1], None,
                        op0=mybir.AluOpType.is_equal,
                    )
                    nc.tensor.matmul(
                        ps[:, c0:c1], gxt[:, stripe, :], s_t[:, 0:wc],
                        start=first, stop=last, skip_group_check=True,
                    )
                retire_q.append((w, ps))
                if len(retire_q) > 1:
                    retire(*retire_q.pop(0))
            while retire_q:
                retire(*retire_q.pop(0))

    nc.compile()
    return nc


def kernel(edge_lists, node_states, W, b):
    edge_lists = np.asarray(edge_lists)
    node_states = np.asarray(node_states, dtype=np.float32)
    W = np.asarray(W, dtype=np.float32)
    b = np.asarray(b, dtype=np.float32)

    sched = build_schedule(edge_lists)
    nc = build_bass(sched)

    x16 = node_states.astype(np.float16)
    iota = np.tile(np.arange(WD, dtype=np.float16), (128, 1))
    in_maps = []
    for e in range(E):
        wt16 = np.ascontiguousarray(W[e * D:(e + 1) * D, :].T).astype(np.float16)
        in_maps.append({
            "x": x16,
            "wt": wt16,
            "gidx": sched["gidx"][e],
            "trel": sched["trel"][e],
            "iota": iota,
        })

    global LAST
    res = run_bass_kernel_spmd(nc, in_maps, core_ids=list(range(E)), trace=TRACE)
    LAST = res

    total = np.zeros((N, D), dtype=np.float32)
    for e in range(E):
        total += res.results[e]["msgT"].astype(np.float32).T
    counts_e = sched["counts_e"].astype(np.float32)
    for e in range(E):
        total += np.outer(counts_e[e], b[e * D:(e + 1) * D])
    counts = counts_e.sum(axis=0)
    divisor = np.where(counts == 0.0, 1.0, counts)
    return (total / divisor[:, None]).astype(np.float32)


def selfcheck_schedule(edge_lists, node_states, W, b):
    """Numpy emulation of the device program for schedule validation."""
    sched = build_schedule(np.asarray(edge_lists))
    x16 = np.asarray(node_states, dtype=np.float32).astype(np.float16)
    calls, runs = sched["calls"], sched["runs"]
    total = np.zeros((N, D), dtype=np.float32)
    for e in range(E):
        # emulate gather
        gidx_w = sched["gidx"][e]
        gvals = {}
        for ci, (g, bkt, off, n) in enumerate(calls):
            if n == 0:
                continue
            cols = gidx_w[:16, off // 16:(off + n) // 16]
            idxs = cols.T.reshape(-1)[:n].astype(np.int64)
            rows = x16[bkt * BS + idxs]          # [n, D]
            nst = -(-n // 128)
            buf = np.zeros((128, nst, D), np.float16)
            pos = np.arange(n)
            buf[pos % 128, pos // 128] = rows
            gvals[ci] = buf
        # emulate windows
        msgT = np.zeros((128, N), dtype=np.float32)
        wt16 = np.ascontiguousarray(W[e * D:(e + 1) * D, :].T).astype(np.float16)
        psums = {}
        for r in runs:
            ci, stripe, p0, k, w, col, first, last = r[:8]
            if first:
                psums[w] = np.zeros((128, WD), np.float32)
            gx = gvals[ci][:, stripe, :].astype(np.float32)   # [128, D]
            rel = sched["trel"][e][:, col]                    # [128]
            S = (rel[:, None] == np.arange(WD)[None, :]).astype(np.float32)
            psums[w] += gx.T @ S
        for w, ps in psums.items():
            nwd = min(WD, N - w * WD)
            agg16 = ps.astype(np.float16).astype(np.float32)
            m = (wt16.astype(np.float32).T @ agg16).astype(np.float16)
            msgT[:, w * WD:w * WD + nwd] = m[:, :nwd].astype(np.float32)
        total += msgT.T
    counts_e = sched["counts_e"].astype(np.float32)
    bb = np.asarray(b, dtype=np.float32)
    for e in range(E):
        total += np.outer(counts_e[e], bb[e * D:(e + 1) * D])
    counts = counts_e.sum(axis=0)
    divisor = np.where(counts == 0.0, 1.0, counts)
    return (total / divisor[:, None]).astype(np.float32)
